# revision 16
# baseline (speedup 1.0000x reference)
"""Trainium2 Bass kernel for a dense transformer block (v3).

Problem: B=8, T=2048, DIM=384, 6 heads (hd=64), FFN hidden 768, causal
attention, RMSNorm (eps 1e-6), exact GELU, fp32 I/O.

Sharding: data-parallel over batch B=8 -> one batch element per NeuronCore,
no collectives. Each core runs the full block on its [2048, 384] slice.

Design (engine-balance: ACT ~125us exp stream is the critical path, PE and
DVE fed underneath it; v2 measured 337us with DVE overloaded at 194us and
18 ACT table loads):
  - Attention: S^T layout, Tq=512 chunks, one [128,1024] S tile per
    (feature-pair, ktile) holding both heads of the pair; QK matmuls
    contract K=64 and run row-tiled (tile_position (0,0)/(64,0)) so the
    two heads compute CONCURRENTLY into the two banks; one exp covers
    both. AV keeps the ones-column normalizer (M=65) with the ones FIRST
    so Z lands on PSUM partition 0 (no cross-partition hop for the
    reciprocal/broadcast chain).
  - RMSNorm entirely off ScalarE: sum(x^2) via DVE scalar_tensor_tensor
    accum, rsqrt via the fp32 magic-constant seed + 2 Newton steps on DVE
    int/float ops. ScalarE runs ONLY exp and gelu -> 2 table loads total.
  - Causal diagonals: exp skips the fully-dead prefix (2D strided AP),
    gpsimd memsets zero the prefix in p, and a narrowed DVE multiply with
    a precomputed 0/1 band masks the triangle.
  - PE transposes are regular bf16 matmuls vs identity (N=128, pipelined,
    HAM-warm) into one PSUM bank; single strided DVE copy evacuates all
    3 chunks into fused feature-major tiles.
  - h = x*rsqrt scaling runs on gpsimd (tensor_scalar, 1-input line rate)
    to keep DVE headroom.
  - Emission is chunk-pipelined ascending: attention for chunk ch starts
    after its own K/Q columns; x+o@wo, norm2, second transpose of chunk
    ch overlap attention of chunk ch+1. FFN (gelu-gated) runs as a tail.
  - PSUM: 2 (shared proj/transpose) + 4 (S x2) + 2 (o_even/o_odd) = 8.
"""

import math
import sys

import ml_dtypes
import numpy as np

for _p in ("/opt/trn_rl_repo",):
    if _p not in sys.path:
        sys.path.append(_p)

import concourse.bacc as bacc
import concourse.bass as bass
import concourse.mybir as mybir
import concourse.tile as tile
from concourse.bass import ts
from concourse.bass_utils import run_bass_kernel_spmd
from concourse.masks import make_identity

F32 = mybir.dt.float32
BF16 = mybir.dt.bfloat16
FP8 = mybir.dt.float8e4
I32 = mybir.dt.int32
AF = mybir.ActivationFunctionType
ALU = mybir.AluOpType
DR = mybir.MatmulPerfMode.DoubleRow
EXPB = -2.0794415416798357  # -ln(8): keeps exp() in fp8e4 range; Z scales too

NCORES = 8
T, D, NH, HD, HDIM = 2048, 384, 6, 64, 768
P = 128
SLOT = 128             # per-head V slot: [ones, 63 zeros, v_0..v_63]
VOFF = 64              # v columns live at [VOFF, VOFF+HD); o rows 64-aligned
NT = T // P            # 16 token tiles
ND = D // P            # 3 feature chunks
NHT = HDIM // P        # 6 FFN hidden chunks
CH = 512               # Tq chunk width
NCH = T // CH          # 4
EPS = 1e-6
SCL = 1.0 / math.sqrt(HD)
MAGIC = 0x5F3759DF


def _body(tc, din, out_d):
    nc = tc.nc

    main_cm = tc.tile_pool(name="main", bufs=1)
    main = main_cm.__enter__()

    # ---- big consolidated input tiles (few, large DMAs; split across the
    # two HWDGE queues so x and weights land in parallel) ----
    xbig0 = main.tile([P, 4 * D], F32, tag="xb0", name="xb0")
    xbig1 = main.tile([P, 12 * D], F32, tag="xb1", name="xb1")
    wq_all = main.tile([P, ND * D], BF16, tag="wqa", name="wqa")
    wk_all = main.tile([P, ND * D], BF16, tag="wka", name="wka")
    wv_all = main.tile([P, ND * D], BF16, tag="wva", name="wva")
    wo_all = main.tile([P, ND * D], BF16, tag="woa", name="woa")
    fw1_all = main.tile([P, ND * HDIM], BF16, tag="f1a", name="f1a")
    fw2_all = main.tile([P, NHT * D], BF16, tag="f2a", name="f2a")

    # critical first: x tiles 0-3 as individual transfers (latency of tile 0
    # gates the whole prologue) on sync queue; QKV weights on scalar queue
    for j in range(4):
        nc.sync.dma_start(xbig0[:, ts(j, D)], din["x"][ts(j, P), :])
    nc.scalar.dma_start(
        wk_all[:].rearrange("p (c d) -> p c d", c=ND),
        din["wk"].rearrange("(c p) d -> p c d", p=P))
    nc.scalar.dma_start(
        wq_all[:].rearrange("p (c d) -> p c d", c=ND),
        din["wq"].rearrange("(c p) d -> p c d", p=P))
    nc.scalar.dma_start(
        wv_all[:].rearrange("p (c d) -> p c d", c=ND),
        din["wv"].rearrange("(c p) d -> p c d", p=P))
    nc.sync.dma_start(
        xbig1[:].rearrange("p (j d) -> p j d", j=12),
        din["x"][512:T, :].rearrange("(j p) d -> p j d", p=P))

    # ---- constants ----
    ident = main.tile([P, P], BF16, tag="ident", name="ident")
    make_identity(nc, ident[:])

    # PE warmup spin: ~5us of back-to-back matmuls so HAM un-throttles the
    # clock (1.2 -> 2.4 GHz) before the real pipeline head arrives.
    warm_cm = tc.tile_pool(name="warm", bufs=1, space="PSUM")
    warm = warm_cm.__enter__()
    wtile = warm.tile([P, P], F32, tag="wrm", name="wrm")
    for _ in range(44):
        nc.tensor.matmul(wtile[:], ident[:], ident[:], start=True, stop=True)
    warm_cm.__exit__(None, None, None)
    onesf = main.tile([P, P], F32, tag="onesf", name="onesf")
    nc.gpsimd.memset(onesf[:], 1.0)
    ones_bf = main.tile([1, P], BF16, tag="ones", name="ones")
    nc.vector.tensor_copy(ones_bf[:], onesf[0:1, :])
    magic_t = main.tile([P, 16], I32, tag="magic", name="magic")
    nc.gpsimd.memset(magic_t[:], MAGIC)
    # band[k, c] = 1 iff c - k >= CH (0/1 mask for causal diagonals)
    bandf = main.tile([P, 2 * CH], F32, tag="bandf", name="bandf")
    nc.gpsimd.memset(bandf[:], 1.0)
    nc.gpsimd.affine_select(out=bandf[:], in_=bandf[:],
                            compare_op=ALU.is_ge, fill=0.0,
                            base=-CH, channel_multiplier=-1,
                            pattern=[[1, 2 * CH]])
    band = main.tile([P, 2 * CH], FP8, tag="band", name="band")
    nc.vector.tensor_copy(band[:], bandf[:])

    s1 = main.tile([P, NT], F32, tag="s1", name="s1")
    s1i = main.tile([P, NT], F32, tag="s1i", name="s1i")
    s2 = main.tile([P, NT], F32, tag="s2", name="s2")
    s2i = main.tile([P, NT], F32, tag="s2i", name="s2i")

    # ---- big feature-major tensors (single tiles; chunk c = cols c*T..) ----
    ht = main.tile([P, ND * T], BF16, tag="ht", name="ht")
    qt = main.tile([P, ND * T], BF16, tag="qt", name="qt")
    kt = main.tile([P, ND * T], BF16, tag="kt", name="kt")
    ot = main.tile([P, ND * T], BF16, tag="ot", name="ot")
    h2t = main.tile([P, ND * T], BF16, tag="h2t", name="h2t")
    gt = main.tile([P, NHT * T], BF16, tag="gt", name="gt")

    x_tiles = ([xbig0[:, ts(j, D)] for j in range(4)]
               + [xbig1[:, ts(j, D)] for j in range(12)])
    # fp8 V slots, paired per two adjacent k-tiles for DoubleRow AV:
    # layout [p, (ko=2, h=NH, e=SLOT)]
    vaug = [main.tile([P, 2 * NH * SLOT], FP8, tag=f"va{jp}", name=f"va{jp}")
            for jp in range(NT // 2)]
    # persistent boundary-pair p tiles (dead prefix of the odd k-tile is
    # zeroed ONCE here and never rewritten): layout [p, (ko=2, par=2, CH)]
    pA_t = main.tile([P, 4 * CH], FP8, tag="pA", name="pA")
    pB_t = main.tile([P, 4 * CH], FP8, tag="pB", name="pB")
    nc.gpsimd.memset(pA_t[:], 0.0)
    nc.gpsimd.memset(pB_t[:], 0.0)
    ebias = main.tile([P, 1], F32, tag="ebias", name="ebias")
    nc.gpsimd.memset(ebias[:], EXPB)

    # ---- weight slice views ----
    wq_s = [wq_all[:, ts(c, D)] for c in range(ND)]
    wk_s = [wk_all[:, ts(c, D)] for c in range(ND)]
    wv_s = [wv_all[:, ts(c, D)] for c in range(ND)]
    wo_s = [wo_all[:, ts(c, D)] for c in range(ND)]
    fw1_s = [fw1_all[:, ts(c, HDIM)] for c in range(ND)]
    fw2_s = [fw2_all[:, ts(c, D)] for c in range(NHT)]
    b1_s = main.tile([P, NHT], F32, tag="b1", name="b1")
    b2_row = main.tile([1, D], BF16, tag="b2", name="b2")

    def dma_bulk():
        nc.scalar.dma_start(
            wo_all[:].rearrange("p (c d) -> p c d", c=ND),
            din["wo"].rearrange("(c p) d -> p c d", p=P))
        nc.scalar.dma_start(
            fw1_all[:].rearrange("p (c h) -> p c h", c=ND),
            din["fw1"].rearrange("(c p) h -> p c h", p=P))
        nc.scalar.dma_start(
            fw2_all[:].rearrange("p (c d) -> p c d", c=NHT),
            din["fw2"].rearrange("(c p) d -> p c d", p=P))
        nc.sync.dma_start(b1_s[:], din["fb1"].rearrange("(a b) -> b a", b=P))
        nc.sync.dma_start(b2_row[:], din["fb2"].rearrange("(a b) -> a b", a=1))

    # per-head V slots: ones col 0 (Z -> PSUM row 0), zeros, v at 64:128
    for jp in range(NT // 2):
        nc.gpsimd.memset(vaug[jp][:], 0.0)
        nc.gpsimd.memset(
            vaug[jp][:].rearrange("p (k h e) -> p k h e", k=2, h=NH)
            [:, :, :, 0:1], 1.0)

    # ---- scratch pools ----
    pscr_cm = tc.tile_pool(name="scr", bufs=3)
    pscr = pscr_cm.__enter__()
    prs_cm = tc.tile_pool(name="rsq", bufs=2)
    prs = prs_cm.__enter__()
    patt_cm = tc.tile_pool(name="att", bufs=3)
    patt = patt_cm.__enter__()
    pnrm_cm = tc.tile_pool(name="nrm", bufs=2)
    pnrm = pnrm_cm.__enter__()
    pout_cm = tc.tile_pool(name="out", bufs=3)
    pout = pout_cm.__enter__()

    pj_cm = tc.tile_pool(name="pj", bufs=2, space="PSUM")
    pj = pj_cm.__enter__()

    def rsqrt_quake(s_acc, s_inv, j0, n):
        """s_inv[:, j0:j0+n] = 1/sqrt(s_acc[:, j0:j0+n]/D + EPS) on DVE."""
        tq = prs.tile([P, 16], F32, tag="tq", name="tq")
        sc = prs.tile([P, 16], F32, tag="sc", name="sc")
        y0 = prs.tile([P, 16], F32, tag="y0", name="y0")
        y1 = prs.tile([P, 16], F32, tag="y1", name="y1")
        t_ = tq[:, 0:n]
        nc.vector.tensor_scalar(t_, s_acc[:, j0 : j0 + n], 1.0 / D, EPS,
                                op0=ALU.mult, op1=ALU.add)
        # seed: y0 = bitcast(MAGIC - (bitcast_i32(t) >> 1))
        nc.vector.tensor_scalar(y0[:, 0:n].bitcast(I32), t_.bitcast(I32),
                                1, None, op0=ALU.arith_shift_right)
        nc.vector.scalar_tensor_tensor(
            y0[:, 0:n].bitcast(I32), magic_t[:, 0:n], 0,
            y0[:, 0:n].bitcast(I32), op0=ALU.bypass, op1=ALU.subtract)
        # one Newton step: y <- y * (1.5 - 0.5 * t * y^2)  (~1e-3 rel, fine
        # at the 2e-2 gate; halves the serial DVE latency on the norm chain)
        for src, dst in ((y0, None),):
            out_ap = s_inv[:, j0 : j0 + n] if dst is None else dst[:, 0:n]
            nc.vector.tensor_mul(sc[:, 0:n], src[:, 0:n], src[:, 0:n])
            nc.vector.tensor_mul(sc[:, 0:n], sc[:, 0:n], t_)
            nc.vector.tensor_scalar(sc[:, 0:n], sc[:, 0:n], -0.5, 1.5,
                                    op0=ALU.mult, op1=ALU.add)
            nc.vector.tensor_mul(out_ap, src[:, 0:n], sc[:, 0:n])

    def norm_and_transpose(js, s_acc, s_inv, dst, via_dma=True):
        """RMSNorm (DVE stats + quake rsqrt), DVE scale, PE transpose."""
        for j in js:
            sq = pscr.tile([P, D], F32, tag="sq", name="sq")
            nc.vector.scalar_tensor_tensor(
                sq[:], x_tiles[j][:], 1.0, x_tiles[j][:],
                op0=ALU.mult, op1=ALU.mult,
                accum_out=s_acc[:, j : j + 1])
        rsqrt_quake(s_acc, s_inv, js[0], len(js))
        for j in js:
            hb = pscr.tile([P, D], BF16, tag="hb", name="hb")
            nc.vector.tensor_scalar_mul(hb[:], x_tiles[j][:],
                                        s_inv[:, j : j + 1])
            tp = pj.tile([P, CH], F32, tag="pj", name="tp")
            for c in range(ND):
                nc.tensor.matmul(tp[:, ts(c, P)], hb[:, ts(c, P)], ident[:],
                                 start=True, stop=True)
            dstv = dst[:].rearrange("p (c t) -> p c t", c=ND)[:, :, ts(j, P)]
            if via_dma:
                # contiguous bf16 evac (2x DVE), then DMA-scatter to chunks
                tb = pscr.tile([P, D], BF16, tag="tb", name="tb")
                nc.vector.tensor_copy(tb[:], tp[:, 0:D])
                nc.sync.dma_start(
                    dstv, tb[:].rearrange("p (c t) -> p c t", c=ND))
            else:
                # latency-critical prologue: strided copy, no DMA hop
                nc.vector.tensor_copy(
                    dstv, tp[:, 0:D].rearrange("p (c t) -> p c t", c=ND))

    def produce_kqv(ch, via_dma=True):
        """K^T/Q^T columns + V slots for chunk ch."""
        for w_s, dstt in ((wk_s, kt), (wq_s, qt)):
            for dt in range(ND):
                ps = pj.tile([P, CH], F32, tag="pj", name="kq")
                for c in range(ND):
                    nc.tensor.matmul(
                        ps[:], w_s[c][:, ts(dt, P)],
                        ht[:, c * T + ch * CH : c * T + ch * CH + CH],
                        start=(c == 0), stop=(c == ND - 1))
                nc.vector.tensor_copy(
                    dstt[:, dt * T + ch * CH : dt * T + ch * CH + CH], ps[:])
        for j in range(4 * ch, 4 * ch + 4):
            ps = pj.tile([P, CH], F32, tag="pj", name="v")
            for c in range(ND):
                nc.tensor.matmul(
                    ps[:, 0:D], ht[:, c * T + j * P : c * T + (j + 1) * P],
                    wv_s[c][:], start=(c == 0), stop=(c == ND - 1))
            dstv = (vaug[j // 2][:]
                    .rearrange("p (k h e) -> p k h e", k=2, h=NH)
                    [:, j % 2, :, VOFF : VOFF + HD])
            if via_dma:
                vb = pscr.tile([P, D], FP8, tag="vb", name="vb")
                nc.vector.tensor_copy(vb[:], ps[:, 0:D])
                nc.sync.dma_start(
                    dstv, vb[:].rearrange("p (h e) -> p h e", h=NH))
            else:
                nc.vector.tensor_copy(
                    dstv, ps[:, 0:D].rearrange("p (h e) -> p h e", h=NH))

    def xwo_chunk(ch):
        """x2 = x + o @ wo for chunk ch's token tiles."""
        for j in range(4 * ch, 4 * ch + 4):
            ps = pj.tile([P, CH], F32, tag="pj", name="xo")
            for dt in range(ND):
                nc.tensor.matmul(
                    ps[:, 0:D], ot[:, dt * T + j * P : dt * T + (j + 1) * P],
                    wo_s[dt][:], start=(dt == 0), stop=(dt == ND - 1))
            nc.vector.tensor_add(x_tiles[j][:], ps[:, 0:D], x_tiles[j][:])

    # ---- prologue: norm1 + K/Q/V for chunk 0 (no DMA hops) ----
    norm_and_transpose(range(4), s1, s1i, ht, via_dma=False)

    psS_cm = tc.tile_pool(name="psS", bufs=2, space="PSUM")
    psS = psS_cm.__enter__()
    psO_cm = tc.tile_pool(name="psO", bufs=1, space="PSUM")
    psO = psO_cm.__enter__()

    produce_kqv(0, via_dma=False)
    dma_bulk()

    for ch in range(NCH):
        js = range(4 * ch, 4 * ch + 4)
        # attention for chunk ch; after each head-pair, slot in independent
        # work so no engine FIFO blocks the next chunk's exp stream
        ntk = 4 * (ch + 1)
        npair = ntk // 2
        for dt in range(ND):
            o_e = psO.tile([P, CH], F32, tag="oe", name="oe")
            o_o = psO.tile([P, CH], F32, tag="oo", name="oo")
            for jp in range(npair):
                # p-pair tile: boundary pairs use the persistent pre-zeroed
                # tiles (dead prefix of the odd tile must read as 0)
                if 2 * jp == 4 * ch:
                    pt, dcol = pA_t, 0
                elif 2 * jp == 4 * ch + 2:
                    pt, dcol = pB_t, 2 * P
                else:
                    pt = patt.tile([P, 4 * CH], FP8, tag="p", name="p")
                    dcol = 0
                for half in range(2):
                    k = 2 * jp + half
                    b = k - 4 * ch
                    d = max(0, b) * P
                    po = half * 2 * CH
                    s_ps = psS.tile([P, 2 * CH], F32, tag="s", name="s")
                    nc.tensor.matmul(
                        s_ps[:, d:CH],
                        kt[0:HD, dt * T + k * P : dt * T + (k + 1) * P],
                        qt[0:HD, dt * T + ch * CH + d : dt * T + ch * CH + CH],
                        start=True, stop=True, tile_position=(0, 0))
                    nc.tensor.matmul(
                        s_ps[:, CH + d : 2 * CH],
                        kt[HD:P, dt * T + k * P : dt * T + (k + 1) * P],
                        qt[HD:P, dt * T + ch * CH + d : dt * T + ch * CH + CH],
                        start=True, stop=True, tile_position=(HD, 0))
                    if d == 0:
                        nc.scalar.activation(pt[:, po : po + 2 * CH], s_ps[:],
                                             AF.Exp, bias=ebias[:, 0:1], scale=SCL)
                    else:
                        # skip the fully-dead prefix of each parity half
                        nc.scalar.activation(
                            pt[:, po : po + 2 * CH]
                            .rearrange("p (v q) -> p v q", v=2)[:, :, d:CH],
                            s_ps[:].rearrange("p (v q) -> p v q", v=2)
                            [:, :, d:CH],
                            AF.Exp, bias=ebias[:, 0:1], scale=SCL)
                    if b >= 0:
                        # 128-wide boundary window gets the triangle mask
                        for par in range(2):
                            nc.vector.tensor_mul(
                                pt[:, po + par * CH + d : po + par * CH + d + P],
                                pt[:, po + par * CH + d : po + par * CH + d + P],
                                band[:, CH : CH + P])
                # DoubleRow AV: contract both k-tiles of the pair at once
                vview = vaug[jp][:].rearrange("p (k h e) -> p k h e",
                                              k=2, h=NH)
                pview = pt[:].rearrange("p (k x) -> p k x", k=2)
                for par, o_ps in ((0, o_e), (1, o_o)):
                    nc.tensor.matmul(
                        o_ps[0:SLOT, dcol:CH],
                        vview[:, :, 2 * dt + par, :],
                        pview[:, :, par * CH + dcol : par * CH + CH],
                        start=(jp == 0), stop=(jp == npair - 1),
                        perf_mode=DR)
            # evacuate + normalize both heads (Z on PSUM row 0; o rows 64:128)
            # fused: o_sb = o_ps * bcast(1/Z) straight out of PSUM (one DVE
            # pass instead of copy-then-multiply)
            for par, o_ps in ((0, o_e), (1, o_o)):
                zf = pnrm.tile([P, CH], F32, tag="zf", name="zf")
                nc.vector.reciprocal_approx_fast(zf[0:1, :], o_ps[0:1, :])
                zb = pnrm.tile([P, CH], BF16, tag="zb", name="zb")
                nc.vector.tensor_copy(zb[0:1, :], zf[0:1, :])
                zbb = pnrm.tile([P, CH], BF16, tag="zbb", name="zbb")
                nc.gpsimd.partition_broadcast(zbb[0:SLOT, :], zb[0:1, :])
                o_sb = pnrm.tile([P, CH], BF16, tag="osb", name="osb")
                nc.vector.tensor_mul(o_sb[VOFF:SLOT, :], o_ps[VOFF:SLOT, :],
                                     zbb[VOFF:SLOT, :])
                hp = par * HD
                nc.sync.dma_start(
                    ot[hp : hp + HD, dt * T + ch * CH : dt * T + ch * CH + CH],
                    o_sb[VOFF:SLOT, :])

            # interleave independent work between head-pairs so no engine
            # FIFO backlog ever stalls the exp stream at chunk boundaries:
            # dt=0 -> next chunk's norm1 (feeds produce at dt=1);
            # dt=1 -> next chunk's K/Q/V (ready well before its QKs);
            # dt=2 -> residual + norm2 for an already-finished chunk
            # (slack work, only the FFN tail needs it).
            if dt == 0 and ch < NCH - 1:
                norm_and_transpose(range(4 * ch + 4, 4 * ch + 8), s1, s1i, ht)
            elif dt == 0 and ch == NCH - 1:
                xwo_chunk(ch - 1)
                norm_and_transpose(range(4 * ch - 4, 4 * ch), s2, s2i, h2t)
            elif dt == 1 and ch < NCH - 1:
                produce_kqv(ch + 1)
            elif dt == 2 and 1 <= ch < NCH - 1:
                xwo_chunk(ch - 1)
                norm_and_transpose(range(4 * ch - 4, 4 * ch), s2, s2i, h2t)

    # drain: residual + norm2 for the last chunk
    xwo_chunk(NCH - 1)
    norm_and_transpose(range(4 * NCH - 4, 4 * NCH), s2, s2i, h2t)

    psO_cm.__exit__(None, None, None)
    psS_cm.__exit__(None, None, None)

    # ---- FFN tail ----
    psF_cm = tc.tile_pool(name="psF", bufs=3, space="PSUM")
    psF = psF_cm.__enter__()
    for hti in range(NHT):
        for h2 in range(2):
            g_ps = psF.tile([P, 2 * CH], F32, tag="g", name="g")
            for m in range(2):
                col = (2 * h2 + m) * CH
                for c in range(ND):
                    nc.tensor.matmul(
                        g_ps[:, ts(m, CH)], fw1_s[c][:, ts(hti, P)],
                        h2t[:, c * T + col : c * T + col + CH],
                        start=(c == 0), stop=(c == ND - 1))
            nc.scalar.activation(
                gt[:, hti * T + h2 * 2 * CH : hti * T + (h2 + 1) * 2 * CH],
                g_ps[:], AF.Gelu, bias=b1_s[:, hti : hti + 1])
    for j in range(NT):
        ps = pj.tile([P, CH], F32, tag="pj", name="f2")
        for c in range(NHT):
            nc.tensor.matmul(
                ps[:, 0:D], gt[:, c * T + j * P : c * T + (j + 1) * P],
                fw2_s[c][:], start=(c == 0), stop=False)
        nc.tensor.matmul(ps[:, 0:D], ones_bf[0:1, :], b2_row[0:1, :],
                         start=False, stop=True)
        o_t = pout.tile([P, D], F32, tag="ot", name="otl")
        nc.vector.tensor_add(o_t[:], ps[:, 0:D], x_tiles[j][:])
        nc.sync.dma_start(out_d[ts(j, P), :], o_t[:])

    psF_cm.__exit__(None, None, None)
    pj_cm.__exit__(None, None, None)
    pout_cm.__exit__(None, None, None)
    pnrm_cm.__exit__(None, None, None)
    patt_cm.__exit__(None, None, None)
    prs_cm.__exit__(None, None, None)
    pscr_cm.__exit__(None, None, None)
    main_cm.__exit__(None, None, None)


_CACHE = {}


def _build():
    if "nc" in _CACHE:
        return _CACHE["nc"]
    nc = bacc.Bacc("TRN2", target_bir_lowering=False, debug=False)
    din = {}
    for name, shape, dt_ in (
        ("x", [T, D], F32), ("wq", [D, D], BF16), ("wk", [D, D], BF16),
        ("wv", [D, D], BF16), ("wo", [D, D], BF16), ("fw1", [D, HDIM], BF16),
        ("fb1", [HDIM], F32), ("fw2", [HDIM, D], BF16), ("fb2", [D], BF16),
    ):
        din[name] = nc.dram_tensor(name, shape, dt_, kind="ExternalInput").ap()
    out_d = nc.dram_tensor("out", [T, D], F32, kind="ExternalOutput").ap()
    with tile.TileContext(nc) as tc:
        _body(tc, din, out_d)
    nc.compile()
    _CACHE["nc"] = nc
    return nc


def run(inputs: dict, trace: bool = False):
    """Run on 8 cores; returns (output [8,T,D], BassKernelResults)."""
    nc = _build()
    x = np.ascontiguousarray(inputs["x"], dtype=np.float32)
    ln1 = np.asarray(inputs["ln1_w"], dtype=np.float32)
    ln2 = np.asarray(inputs["ln2_w"], dtype=np.float32)
    shared = {
        "wq": (ln1[:, None] * np.asarray(inputs["wq"], np.float32)).astype(ml_dtypes.bfloat16),
        "wk": (ln1[:, None] * np.asarray(inputs["wk"], np.float32)).astype(ml_dtypes.bfloat16),
        "wv": (ln1[:, None] * np.asarray(inputs["wv"], np.float32)).astype(ml_dtypes.bfloat16),
        "wo": np.asarray(inputs["wo"], np.float32).astype(ml_dtypes.bfloat16),
        "fw1": (ln2[:, None] * np.asarray(inputs["ff_w1"], np.float32)).astype(ml_dtypes.bfloat16),
        "fb1": np.asarray(inputs["ff_b1"], np.float32),
        "fw2": np.asarray(inputs["ff_w2"], np.float32).astype(ml_dtypes.bfloat16),
        "fb2": np.asarray(inputs["ff_b2"], np.float32).astype(ml_dtypes.bfloat16),
    }
    shared = {k: np.ascontiguousarray(v) for k, v in shared.items()}
    in_maps = [dict(shared, x=np.ascontiguousarray(x[c])) for c in range(NCORES)]
    res = run_bass_kernel_spmd(nc, in_maps, list(range(NCORES)), trace=trace)
    out = np.stack([res.results[c]["out"] for c in range(NCORES)], axis=0)
    return out, res


def kernel(**inputs) -> np.ndarray:
    out, _ = run(inputs, trace=False)
    return out

